# revision 8
# baseline (speedup 1.0000x reference)
"""Trainium2 Bass kernel for nn_Adj (topk_masking).

Computes, per batch b:
    si   = x_b @ x_b^T                      (512, 512)
    th_i = 32nd largest value of row i
    adj  = (si >= th)                       row degree == 32 (no boundary ties)
    out  = adj / 32                         (D^-1/2 A D^-1/2 with D = 32 I)

Sharding: pure data-parallel over batch; core i handles batches [8i, 8i+8).
The host pre-transposes x so each core receives x_b^T (C-major), which is
what the PE needs for both matmul operands (si = lhsT.T @ rhs with
lhsT = rhs = x_b^T); no on-chip transposes of the input.

si is symmetric, so only the upper-triangular 128-row x 128-col blocks are
computed by matmul (62.5% of the FLOPs); the lower blocks are mirrored with
PE transposes. Top-32 per row via DVE max8/match_replace8; the final
mask+scale runs on the (otherwise idle) GPSIMD engine.
"""

import os
import sys

import numpy as np


def _import_concourse():
    try:
        import concourse.bass  # noqa: F401
        return
    except ImportError:
        pass
    for p in ("/opt/trn_rl_repo", "/root/.axon_site/_ro/trn_rl_repo"):
        if os.path.isdir(p) and p not in sys.path:
            sys.path.insert(0, p)
    import concourse.bass  # noqa: F401


B, N, C = 64, 512, 1024
K = 32
NCORES = 8
BPC = B // NCORES  # batches per core
P = 128            # SBUF partitions
KT = C // P        # contraction tiles per batch
MT = N // P        # output row tiles per batch
NEG = -1.0e30      # replacement sentinel, far below any |si| value

# "f32_tri": exact fp32 matmul on upper-triangular blocks + mirrored
# transposes. "f32r_full": full square matmul in f32r (PE full rate but
# ~11-bit input mantissa: ~300 boundary flips, rel err ~1.7e-2).
MODE = os.environ.get("ADJ_MODE", "f32_tri")


def _build_nc(mode=MODE):
    _import_concourse()
    import concourse.bacc as bacc
    import concourse.mybir as mybir
    from concourse.masks import make_identity
    from concourse.tile import TileContext

    tri = mode.endswith("_tri")
    fr = mybir.dt.float32r if mode.startswith("f32r") else mybir.dt.float32

    nc = bacc.Bacc("TRN2", target_bir_lowering=False)
    xt = nc.dram_tensor("xt", [BPC, C, N], fr, kind="ExternalInput")
    out = nc.dram_tensor("out", [BPC, N, N], mybir.dt.float32, kind="ExternalOutput")

    with TileContext(nc) as tc:
        with (
            tc.tile_pool(name="xtp", bufs=2) as xtp,
            tc.tile_pool(name="psp", bufs=4, space="PSUM") as psp,
            tc.tile_pool(name="tpp", bufs=3, space="PSUM") as tpp,
            tc.tile_pool(name="sip", bufs=8) as sip,
            tc.tile_pool(name="wrkp", bufs=4) as wrkp,
            tc.tile_pool(name="v8p", bufs=16) as v8p,
            tc.tile_pool(name="mp", bufs=6) as mp,
            tc.tile_pool(name="cstp", bufs=1) as cstp,
        ):
            ident = None
            if tri:
                ident = cstp.tile([P, P], mybir.dt.float32)
                make_identity(nc, ident)
            for b in range(BPC):
                xtb = xtp.tile([P, KT, N], fr)
                for k in range(KT):
                    nc.sync.dma_start(
                        out=xtb[:, k, :],
                        in_=xt[b, k * P:(k + 1) * P, :],
                    )
                sis = []
                for m in range(MT):
                    c0 = m * P if tri else 0  # first computed column
                    ps = psp.tile([P, N], mybir.dt.float32)
                    for k in range(KT):
                        nc.tensor.matmul(
                            ps[:, c0:],
                            lhsT=xtb[:, k, m * P:(m + 1) * P],
                            rhs=xtb[:, k, c0:],
                            start=(k == 0),
                            stop=(k == KT - 1),
                        )
                    si = sip.tile([P, N], mybir.dt.float32)
                    sis.append(si)
                    nc.scalar.copy(si[:, c0:], ps[:, c0:])
                    if tri:
                        for j in range(m):
                            # block (m, j) = block (j, m)^T
                            pst = tpp.tile([P, P], mybir.dt.float32)
                            nc.tensor.transpose(
                                pst,
                                in_=sis[j][:, m * P:(m + 1) * P],
                                identity=ident,
                            )
                            nc.scalar.copy(si[:, j * P:(j + 1) * P], pst)
                    # top-32 per row: 4 rounds of max8, removing each round's
                    # 8 winners; round 4's minimum is the 32nd largest.
                    wrk = wrkp.tile([P, N], mybir.dt.float32)
                    src = si
                    v8 = None
                    for r in range(4):
                        v8 = v8p.tile([P, 8], mybir.dt.float32)
                        nc.vector.max(out=v8, in_=src)
                        if r < 3:
                            nc.vector.match_replace(
                                out=wrk, in_to_replace=v8, in_values=src,
                                imm_value=NEG,
                            )
                            src = wrk
                    msk = mp.tile([P, N], mybir.dt.float32)
                    nc.gpsimd.tensor_scalar(
                        out=msk, in0=si, scalar1=v8[:, 7:8], scalar2=1.0 / K,
                        op0=mybir.AluOpType.is_ge, op1=mybir.AluOpType.mult,
                    )
                    nc.sync.dma_start(out=out[b, m * P:(m + 1) * P, :], in_=msk)
    nc.compile()
    return nc


_NC_CACHE = {}


def _get_nc(mode=MODE):
    if mode not in _NC_CACHE:
        _NC_CACHE[mode] = _build_nc(mode)
    return _NC_CACHE[mode]


def _run(xt, mode=MODE, trace=False):
    """xt: (B, C, N) float32, batch-transposed input. Returns (results, out)."""
    _import_concourse()
    from concourse.bass_utils import run_bass_kernel_spmd

    nc = _get_nc(mode)
    in_maps = [
        {"xt": np.ascontiguousarray(xt[i * BPC:(i + 1) * BPC])}
        for i in range(NCORES)
    ]
    res = run_bass_kernel_spmd(nc, in_maps, core_ids=list(range(NCORES)),
                               trace=trace)
    out = np.concatenate([res.results[i]["out"] for i in range(NCORES)], axis=0)
    return res, out


def kernel(x):
    x = np.asarray(x, dtype=np.float32)
    xt = np.ascontiguousarray(x.transpose(0, 2, 1))  # (B, C, N)
    _, out = _run(xt)
    return out


# revision 9
# speedup vs baseline: 1.4394x; 1.4394x over previous
"""Trainium2 Bass kernel for nn_Adj (topk_masking).

Computes, per batch b:
    si   = x_b @ x_b^T                      (512, 512)
    th_i = 32nd largest value of row i
    adj  = (si >= th)                       row degree == 32 (no boundary ties)
    out  = adj / 32                         (D^-1/2 A D^-1/2 with D = 32 I)

Sharding: pure data-parallel over batch; core i handles batches [8i, 8i+8).
The host pre-transposes x so each core receives x_b^T (C-major), which is
what the PE needs for both matmul operands (si = lhsT.T @ rhs with
lhsT = rhs = x_b^T); no on-chip transposes of the input.

si is symmetric, so only the upper-triangular 128-row x 128-col blocks are
computed by matmul (62.5% of the FLOPs); the lower blocks are mirrored with
PE transposes. Top-32 per row via DVE max8/match_replace8; the final
mask+scale runs on the (otherwise idle) GPSIMD engine.
"""

import os
import sys

import numpy as np


def _import_concourse():
    try:
        import concourse.bass  # noqa: F401
        return
    except ImportError:
        pass
    for p in ("/opt/trn_rl_repo", "/root/.axon_site/_ro/trn_rl_repo"):
        if os.path.isdir(p) and p not in sys.path:
            sys.path.insert(0, p)
    import concourse.bass  # noqa: F401


B, N, C = 64, 512, 1024
K = 32
NCORES = 8
BPC = B // NCORES  # batches per core
P = 128            # SBUF partitions
KT = C // P        # contraction tiles per batch
MT = N // P        # output row tiles per batch
NEG = -1.0e30      # replacement sentinel, far below any |si| value

# "f32_tri": exact fp32 matmul on upper-triangular blocks + mirrored
# transposes. "f32r_full": full square matmul in f32r (PE full rate but
# ~11-bit input mantissa: ~300 boundary flips, rel err ~1.7e-2).
MODE = os.environ.get("ADJ_MODE", "f32_tri")


def _build_nc(mode=MODE):
    _import_concourse()
    import concourse.bacc as bacc
    import concourse.mybir as mybir
    from concourse.masks import make_identity
    from concourse.tile import TileContext

    tri = mode.endswith("_tri")
    fr = mybir.dt.float32r if mode.startswith("f32r") else mybir.dt.float32

    nc = bacc.Bacc("TRN2", target_bir_lowering=False)
    xt = nc.dram_tensor("xt", [BPC, C, N], fr, kind="ExternalInput")
    out = nc.dram_tensor("out", [BPC, N, N], mybir.dt.float32, kind="ExternalOutput")

    with TileContext(nc) as tc:
        with (
            tc.tile_pool(name="xtp", bufs=2) as xtp,
            tc.tile_pool(name="psp", bufs=4, space="PSUM") as psp,
            tc.tile_pool(name="tpp", bufs=3, space="PSUM") as tpp,
            tc.tile_pool(name="sip", bufs=8) as sip,
            tc.tile_pool(name="wrkp", bufs=4) as wrkp,
            tc.tile_pool(name="v8p", bufs=16) as v8p,
            tc.tile_pool(name="mp", bufs=6) as mp,
            tc.tile_pool(name="cstp", bufs=1) as cstp,
        ):
            ident = None
            if tri:
                ident = cstp.tile([P, P], mybir.dt.float32)
                make_identity(nc, ident)
            for b in range(BPC):
                xtb = xtp.tile([P, KT, N], fr)
                for k in range(KT):
                    nc.sync.dma_start(
                        out=xtb[:, k, :],
                        in_=xt[b, k * P:(k + 1) * P, :],
                    )
                sis = []
                for m in range(MT):
                    c0 = m * P if tri else 0  # first computed column
                    ps = psp.tile([P, N], mybir.dt.float32)
                    for k in range(KT):
                        nc.tensor.matmul(
                            ps[:, c0:],
                            lhsT=xtb[:, k, m * P:(m + 1) * P],
                            rhs=xtb[:, k, c0:],
                            start=(k == 0),
                            stop=(k == KT - 1),
                        )
                    si = sip.tile([P, N], mybir.dt.float32)
                    sis.append(si)
                    nc.scalar.copy(si[:, c0:], ps[:, c0:])
                    if tri:
                        for j in range(m):
                            # block (m, j) = block (j, m)^T
                            pst = tpp.tile([P, P], mybir.dt.float32)
                            nc.tensor.transpose(
                                pst,
                                in_=sis[j][:, m * P:(m + 1) * P],
                                identity=ident,
                            )
                            nc.scalar.copy(si[:, j * P:(j + 1) * P], pst)
                    # top-32 per row: 4 rounds of max8, removing each round's
                    # 8 winners; round 4's minimum is the 32nd largest.
                    wrk = wrkp.tile([P, N], mybir.dt.float32)
                    src = si
                    v8 = None
                    for r in range(4):
                        v8 = v8p.tile([P, 8], mybir.dt.float32)
                        nc.vector.max(out=v8, in_=src)
                        if r < 3:
                            nc.vector.match_replace(
                                out=wrk, in_to_replace=v8, in_values=src,
                                imm_value=NEG,
                            )
                            src = wrk
                    msk = mp.tile([P, N], mybir.dt.float32)
                    nc.vector.tensor_scalar(
                        out=msk, in0=si, scalar1=v8[:, 7:8], scalar2=1.0 / K,
                        op0=mybir.AluOpType.is_ge, op1=mybir.AluOpType.mult,
                    )
                    nc.sync.dma_start(out=out[b, m * P:(m + 1) * P, :], in_=msk)
    nc.compile()
    return nc


_NC_CACHE = {}


def _get_nc(mode=MODE):
    if mode not in _NC_CACHE:
        _NC_CACHE[mode] = _build_nc(mode)
    return _NC_CACHE[mode]


def _run(xt, mode=MODE, trace=False):
    """xt: (B, C, N) float32, batch-transposed input. Returns (results, out)."""
    _import_concourse()
    from concourse.bass_utils import run_bass_kernel_spmd

    nc = _get_nc(mode)
    in_maps = [
        {"xt": np.ascontiguousarray(xt[i * BPC:(i + 1) * BPC])}
        for i in range(NCORES)
    ]
    res = run_bass_kernel_spmd(nc, in_maps, core_ids=list(range(NCORES)),
                               trace=trace)
    out = np.concatenate([res.results[i]["out"] for i in range(NCORES)], axis=0)
    return res, out


def kernel(x):
    x = np.asarray(x, dtype=np.float32)
    xt = np.ascontiguousarray(x.transpose(0, 2, 1))  # (B, C, N)
    _, out = _run(xt)
    return out
